# revision 1
# baseline (speedup 1.0000x reference)
"""DGL-life GCN classifier on 8 Trainium2 NeuronCores (Bass/Tile).

Strategy: shard the batched graph by dst-node across 8 cores (cuts aligned to
graph boundaries).  Each core holds a full replica of the current layer's
node features in HBM (bf16).  Per layer: per-edge rows are fetched with
indirect DMA gathers (128 rows / instruction), segment-summed into 128-dst
blocks via one-hot matmuls accumulated in PSUM, then the dense GraphConv /
residual transforms run in the transposed domain where the per-output-feature
bias rides the ACT relu for free.  Feature shards are exchanged between
layers with an AllGather collective.  SumPooling reuses the same one-hot
matmul machinery over graph ids, followed by the 2-layer MLP classifier.
"""
import sys
sys.path.insert(0, "/opt/trn_rl_repo")

import numpy as np
import ml_dtypes

bf16 = ml_dtypes.bfloat16

N_NODES = 500000
N_EDGES = 8000000
N_GRAPHS = 16384
IN_F = 74
HID = 64
CLS_H = 128
N_CLASSES = 2

NCORES = 8
NPAD = 62976          # padded nodes per shard (492 blocks of 128)
NBLK = 492
GRP = 4               # dst blocks per For_i group
NGRP = NBLK // GRP    # 82
EBLK = 2304           # edge-slot capacity per dst block (18 tiles of 128)
TPB = EBLK // 128     # 18
COLS = GRP * TPB      # 108 gather columns per group
NTOT = NCORES * NPAD  # 503808 padded global rows
GPAD = 2176           # padded graphs per shard (17 blocks of 128)
NGB = GPAD // 128     # 17
PT = 33               # pooling node tiles per graph block

_cache = {}


def _host_prep(node_feats, src, dst, graph_ids):
    gid = np.asarray(graph_ids)
    node_of_graph_start = np.searchsorted(gid, np.arange(N_GRAPHS))
    cuts = [0]
    for k in range(1, NCORES):
        target = k * N_NODES // NCORES
        gi = np.searchsorted(node_of_graph_start, target)
        cand = [node_of_graph_start[min(gi, N_GRAPHS - 1)],
                node_of_graph_start[max(gi - 1, 0)]]
        cuts.append(int(min(cand, key=lambda x: abs(x - target))))
    cuts.append(N_NODES)
    cuts = np.asarray(cuts, np.int64)
    shard_sizes = np.diff(cuts)
    assert shard_sizes.max() <= NPAD

    gstart = [int(gid[c]) if c < N_NODES else N_GRAPHS for c in cuts[:-1]] + [N_GRAPHS]
    gstart = np.asarray(gstart, np.int64)
    assert np.diff(gstart).max() <= GPAD

    src = np.asarray(src).astype(np.int64)
    dst = np.asarray(dst).astype(np.int64)
    shard_of_src = np.searchsorted(cuts, src, side="right") - 1
    src_pad = shard_of_src * NPAD + (src - cuts[shard_of_src])

    # padded global bf16 feature table for layer 0
    table0 = np.zeros((NTOT, IN_F), np.float32)
    for k in range(NCORES):
        n = cuts[k + 1] - cuts[k]
        table0[k * NPAD:k * NPAD + n] = node_feats[cuts[k]:cuts[k + 1]]

    per_core = []
    shard_of_dst = np.searchsorted(cuts, dst, side="right") - 1
    for k in range(NCORES):
        base, n = cuts[k], cuts[k + 1] - cuts[k]
        m = shard_of_dst == k
        e_src = src_pad[m]
        e_dst = dst[m] - base
        order = np.argsort(e_dst, kind="stable")
        e_src, e_dst = e_src[order], e_dst[order]
        blk = e_dst >> 7
        cnt = np.bincount(blk, minlength=NBLK)
        assert cnt.max() <= EBLK, cnt.max()
        cum = np.concatenate([[0], np.cumsum(cnt)])
        slot = np.arange(len(e_dst)) - cum[blk]
        idx_arr = np.zeros((NBLK, EBLK), np.int32)
        dst_arr = np.full((NBLK, EBLK), -1.0, np.float32)
        idx_arr[blk, slot] = e_src.astype(np.int32)
        dst_arr[blk, slot] = (e_dst & 127).astype(np.float32)
        # [NBLK, TPB, 128] -> [128, NGRP, GRP*TPB]
        eidx = (idx_arr.reshape(NGRP, GRP, TPB, 128)
                .transpose(3, 0, 1, 2).reshape(128, NGRP, COLS).copy())
        edst = (dst_arr.reshape(NGRP, GRP, TPB, 128)
                .transpose(3, 0, 1, 2).reshape(128, NGRP, COLS).copy())

        # pooling: local nodes sorted by graph; graph-block-aligned slots
        gl = gid[base:base + n] - gstart[k]          # local graph id per node
        gb = gl >> 7
        pcnt = np.bincount(gb, minlength=NGB)
        assert pcnt.max() <= PT * 128
        pcum = np.concatenate([[0], np.cumsum(pcnt)])
        pslot = np.arange(n) - pcum[gb]
        pidx_arr = np.zeros((NGB, PT * 128), np.int32)
        pdst_arr = np.full((NGB, PT * 128), -1.0, np.float32)
        pidx_arr[gb, pslot] = np.arange(n, dtype=np.int32)
        pdst_arr[gb, pslot] = (gl & 127).astype(np.float32)
        pidx = (pidx_arr.reshape(NGB, PT, 128)
                .transpose(2, 0, 1).reshape(128, NGB, PT).copy())
        pdst = (pdst_arr.reshape(NGB, PT, 128)
                .transpose(2, 0, 1).reshape(128, NGB, PT).copy())

        h0T = np.zeros((IN_F, NPAD), np.float32)
        h0T[:, :n] = node_feats[base:base + n].astype(np.float32).T

        per_core.append(dict(eidx=eidx, edst=edst, pidx=pidx, pdst=pdst, h0T=h0T))
    return cuts, gstart, table0, per_core


def _build_nc():
    import concourse.bass as bass
    from concourse import bacc
    import concourse.mybir as mybir
    import concourse.tile as tile

    fp32 = mybir.dt.float32
    b16 = mybir.dt.bfloat16

    nc = bacc.Bacc("TRN2", target_bir_lowering=False, debug=False,
                   num_devices=NCORES)

    table0 = nc.dram_tensor("table0", [NTOT, IN_F], fp32, kind="ExternalInput")
    h0T_in = nc.dram_tensor("h0T", [IN_F, NPAD], fp32, kind="ExternalInput")
    eidx_in = nc.dram_tensor("eidx", [128, NGRP, COLS], mybir.dt.int32, kind="ExternalInput")
    edst_in = nc.dram_tensor("edst", [128, NGRP, COLS], fp32, kind="ExternalInput")
    pidx_in = nc.dram_tensor("pidx", [128, NGB, PT], mybir.dt.int32, kind="ExternalInput")
    pdst_in = nc.dram_tensor("pdst", [128, NGB, PT], fp32, kind="ExternalInput")
    iota_in = nc.dram_tensor("iota", [128, 128], fp32, kind="ExternalInput")
    ident_in = nc.dram_tensor("ident", [128, 128], fp32, kind="ExternalInput")
    gW_in = [nc.dram_tensor(f"gW{i}", [IN_F if i == 0 else HID, HID], fp32, kind="ExternalInput") for i in range(3)]
    rW_in = [nc.dram_tensor(f"rW{i}", [IN_F if i == 0 else HID, HID], fp32, kind="ExternalInput") for i in range(3)]
    gb_in = [nc.dram_tensor(f"gb{i}", [HID, 1], fp32, kind="ExternalInput") for i in range(3)]
    rb_in = [nc.dram_tensor(f"rb{i}", [HID, 1], fp32, kind="ExternalInput") for i in range(3)]
    cW1_in = nc.dram_tensor("cW1", [HID, CLS_H], fp32, kind="ExternalInput")
    cb1_in = nc.dram_tensor("cb1", [CLS_H, 1], fp32, kind="ExternalInput")
    cW2_in = nc.dram_tensor("cW2", [CLS_H, N_CLASSES], fp32, kind="ExternalInput")
    cb2_in = nc.dram_tensor("cb2t", [N_CLASSES, 128], fp32, kind="ExternalInput")
    logits_out = nc.dram_tensor("logitsT", [N_CLASSES, GPAD], fp32, kind="ExternalOutput")

    with tile.TileContext(nc) as tc:
        with (
            tc.tile_pool(name="const", bufs=1) as constp,
            tc.tile_pool(name="persist", bufs=1) as persistp,
            tc.tile_pool(name="meta", bufs=2) as metap,
            tc.tile_pool(name="slab", bufs=2) as slabp,
            tc.tile_pool(name="p1", bufs=4) as pp,
            tc.tile_pool(name="sb", bufs=4) as sbp,
            tc.tile_pool(name="agg_ps", bufs=2, space="PSUM") as aggps,
            tc.tile_pool(name="mm_ps", bufs=2, space="PSUM") as mmps,
            tc.tile_pool(name="tp_ps", bufs=2, space="PSUM") as tpps,
            tc.tile_pool(name="dram", bufs=1, space="DRAM") as dramp,
        ):
            iota = constp.tile([128, 128], fp32)
            nc.sync.dma_start(iota[:], iota_in[:])
            ident = constp.tile([128, 128], fp32)
            nc.sync.dma_start(ident[:], ident_in[:])
            gW = []
            rW = []
            gb = []
            rb = []
            for i in range(3):
                kdim = IN_F if i == 0 else HID
                w1 = constp.tile([kdim, HID], fp32)
                nc.sync.dma_start(w1[:], gW_in[i][:])
                gW.append(w1)
                w2 = constp.tile([kdim, HID], fp32)
                nc.sync.dma_start(w2[:], rW_in[i][:])
                rW.append(w2)
                t1 = constp.tile([HID, 1], fp32)
                nc.sync.dma_start(t1[:], gb_in[i][:])
                gb.append(t1)
                t2 = constp.tile([HID, 1], fp32)
                nc.sync.dma_start(t2[:], rb_in[i][:])
                rb.append(t2)
            cW1 = constp.tile([HID, CLS_H], fp32)
            nc.sync.dma_start(cW1[:], cW1_in[:])
            cb1 = constp.tile([CLS_H, 1], fp32)
            nc.sync.dma_start(cb1[:], cb1_in[:])
            cW2 = constp.tile([CLS_H, N_CLASSES], fp32)
            nc.sync.dma_start(cW2[:], cW2_in[:])
            cb2 = constp.tile([N_CLASSES, 128], fp32)
            nc.sync.dma_start(cb2[:], cb2_in[:])

            

            cc_in = [dramp.tile([NPAD, HID], fp32, name=f"cc_in{i}") for i in range(2)]
            hT_dram = dramp.tile([HID, NPAD], fp32, name="hT_dram")
            cc_out = [dramp.tile([NTOT, HID], fp32, addr_space="Shared", name=f"cc_out{i}") for i in range(2)]
            h3_local = dramp.tile([NPAD, HID], fp32)

            Relu = nc.mybir.ActivationFunctionType.Relu if hasattr(nc, "mybir") else None
            import concourse.mybir as mybir2
            Relu = mybir2.ActivationFunctionType.Relu
            EQ = mybir2.AluOpType.is_equal

            for L in range(3):
                DIN = IN_F if L == 0 else HID
                table_ap = table0[:] if L == 0 else cc_out[L - 1][:]
                with tc.For_i(0, NGRP, 1) as g:
                    idx_t = metap.tile([128, COLS], mybir.dt.int32)
                    nc.sync.dma_start(idx_t[:], eidx_in[:, bass.ds(g, 1), :])
                    dst_t = metap.tile([128, COLS], fp32)
                    nc.sync.dma_start(dst_t[:], edst_in[:, bass.ds(g, 1), :])
                    slab = slabp.tile([128, COLS, IN_F], fp32, tag="slab", name="slab")[:, :, :DIN]
                    hTg = slabp.tile([IN_F, GRP * 128], fp32, tag="hTg", name="hTg")[:DIN, :]
                    hT_src = h0T_in if L == 0 else hT_dram
                    nc.sync.dma_start(hTg[:], hT_src[:DIN, bass.ts(g, GRP * 128)])
                    for j in range(COLS):
                        nc.gpsimd.indirect_dma_start(
                            out=slab[:, j, :], out_offset=None, in_=table_ap,
                            in_offset=bass.IndirectOffsetOnAxis(
                                ap=idx_t[:, j:j + 1], axis=0))
                    for b6 in range(GRP):
                        psum = aggps.tile([DIN, 128], fp32, space="PSUM", tag="agg")
                        for t in range(TPB):
                            j = b6 * TPB + t
                            p_t = pp.tile([128, 128], fp32)
                            nc.vector.tensor_tensor(
                                out=p_t[:],
                                in0=dst_t[:, j:j + 1].to_broadcast([128, 128]),
                                in1=iota[:], op=EQ)
                            nc.tensor.matmul(out=psum[:], lhsT=slab[:, j, :],
                                             rhs=p_t[:], start=(t == 0),
                                             stop=(t == TPB - 1))
                        aggT = sbp.tile([DIN, 128], fp32, tag="aggT")
                        nc.vector.tensor_copy(aggT[:], psum[:])
                        convp = mmps.tile([HID, 128], fp32, space="PSUM", tag="conv")
                        nc.tensor.matmul(out=convp[:], lhsT=gW[L][:], rhs=aggT[:],
                                         start=True, stop=True)
                        hTb = hTg[:, b6 * 128:(b6 + 1) * 128]
                        resp = mmps.tile([HID, 128], fp32, space="PSUM", tag="res")
                        nc.tensor.matmul(out=resp[:], lhsT=rW[L][:], rhs=hTb,
                                         start=True, stop=True)
                        convs = sbp.tile([HID, 128], fp32, tag="convs")
                        nc.scalar.activation(convs[:], convp[:], Relu, bias=gb[L][:, :1])
                        ress = sbp.tile([HID, 128], fp32, tag="ress")
                        nc.scalar.activation(ress[:], resp[:], Relu, bias=rb[L][:, :1])
                        hnewT = sbp.tile([HID, 128], fp32, tag="hnewT")
                        nc.vector.tensor_add(hnewT[:], convs[:], ress[:])
                        if L < 2:
                            nc.sync.dma_start(
                                hT_dram[:, bass.ts(g, GRP * 128)][:, b6 * 128:(b6 + 1) * 128],
                                hnewT[:])
                        tp = tpps.tile([128, HID], fp32, space="PSUM", tag="tp")
                        nc.tensor.transpose(out=tp[:], in_=hnewT[:],
                                            identity=ident[:HID, :HID])
                        hnew = sbp.tile([128, HID], fp32, tag="hnew")
                        nc.vector.tensor_copy(hnew[:], tp[:])
                        dst_dram = cc_in[L] if L < 2 else h3_local
                        dd = dst_dram[:].rearrange("(g x) d -> g x d", x=GRP * 128)
                        nc.sync.dma_start(
                            dd[bass.ds(g, 1), b6 * 128:(b6 + 1) * 128, :], hnew[:])
                if L < 2:
                    nc.gpsimd.collective_compute(
                        "AllGather", mybir2.AluOpType.bypass,
                        replica_groups=[list(range(NCORES))],
                        ins=[cc_in[L][:].opt()], outs=[cc_out[L][:].opt()])

            # -------- pooling + classifier --------
            out_slab = persistp.tile([N_CLASSES, GPAD], fp32)
            with tc.For_i(0, NGB, 1) as gbv:
                pidx_t = metap.tile([128, PT], mybir.dt.int32, tag="pidx")
                nc.sync.dma_start(pidx_t[:], pidx_in[:, bass.ds(gbv, 1), :])
                pdst_t = metap.tile([128, PT], fp32, tag="pdst")
                nc.sync.dma_start(pdst_t[:], pdst_in[:, bass.ds(gbv, 1), :])
                pslab = slabp.tile([128, PT, HID], fp32, tag="pslab")
                for t in range(PT):
                    nc.gpsimd.indirect_dma_start(
                        out=pslab[:, t, :], out_offset=None, in_=h3_local[:],
                        in_offset=bass.IndirectOffsetOnAxis(
                            ap=pidx_t[:, t:t + 1], axis=0))
                gpsum = aggps.tile([HID, 128], fp32, space="PSUM", tag="agg", name="gpsum")
                for t in range(PT):
                    p_t = pp.tile([128, 128], fp32, tag="pp")
                    nc.vector.tensor_tensor(
                        out=p_t[:], in0=pdst_t[:, t:t + 1].to_broadcast([128, 128]),
                        in1=iota[:], op=EQ)
                    nc.tensor.matmul(out=gpsum[:], lhsT=pslab[:, t, :], rhs=p_t[:],
                                     start=(t == 0), stop=(t == PT - 1))
                graphT = sbp.tile([HID, 128], fp32, tag="graphT")
                nc.vector.tensor_copy(graphT[:], gpsum[:])
                hidp = mmps.tile([CLS_H, 128], fp32, space="PSUM", tag="conv", name="hidp")
                nc.tensor.matmul(out=hidp[:], lhsT=cW1[:], rhs=graphT[:],
                                 start=True, stop=True)
                hid = sbp.tile([CLS_H, 128], fp32, tag="hids")
                nc.scalar.activation(hid[:], hidp[:], Relu, bias=cb1[:, :1])
                logp = tpps.tile([N_CLASSES, 128], fp32, space="PSUM", tag="tp", name="logp")
                nc.tensor.matmul(out=logp[:], lhsT=cW2[:], rhs=hid[:],
                                 start=True, stop=True)
                nc.vector.tensor_add(
                    out_slab[:, bass.ts(gbv, 128)], logp[:], cb2[:])
            nc.sync.dma_start(logits_out[:], out_slab[:])

    nc.compile()
    return nc


def kernel(node_feats, src, dst, graph_ids,
           gW0, gb0, rW0, rb0, gW1, gb1, rW1, rb1, gW2, gb2, rW2, rb2,
           cW1, cb1, cW2, cb2):
    from concourse.bass_utils import run_bass_kernel_spmd

    node_feats = np.asarray(node_feats)
    cuts, gstart, table0, per_core = _host_prep(node_feats, src, dst, graph_ids)

    if "nc" not in _cache:
        _cache["nc"] = _build_nc()
    nc = _cache["nc"]

    iota = np.tile(np.arange(128, dtype=np.float32), (128, 1))
    ident = np.eye(128, dtype=np.float32)
    common = {
        "table0": table0,
        "iota": np.asarray(iota), "ident": np.asarray(ident),
        "gW0": np.asarray(gW0, np.float32), "rW0": np.asarray(rW0, np.float32),
        "gW1": np.asarray(gW1, np.float32), "rW1": np.asarray(rW1, np.float32),
        "gW2": np.asarray(gW2, np.float32), "rW2": np.asarray(rW2, np.float32),
        "gb0": np.asarray(gb0, np.float32).reshape(HID, 1),
        "gb1": np.asarray(gb1, np.float32).reshape(HID, 1),
        "gb2": np.asarray(gb2, np.float32).reshape(HID, 1),
        "rb0": np.asarray(rb0, np.float32).reshape(HID, 1),
        "rb1": np.asarray(rb1, np.float32).reshape(HID, 1),
        "rb2": np.asarray(rb2, np.float32).reshape(HID, 1),
        "cW1": np.asarray(cW1, np.float32),
        "cb1": np.asarray(cb1, np.float32).reshape(CLS_H, 1),
        "cW2": np.asarray(cW2, np.float32),
        "cb2t": np.tile(np.asarray(cb2, np.float32).reshape(N_CLASSES, 1), (1, 128)),
    }
    in_maps = []
    for k in range(NCORES):
        m = dict(common)
        m["h0T"] = per_core[k]["h0T"]
        m["eidx"] = per_core[k]["eidx"]
        m["edst"] = per_core[k]["edst"]
        m["pidx"] = per_core[k]["pidx"]
        m["pdst"] = per_core[k]["pdst"]
        in_maps.append(m)

    import time as _time
    _t0 = _time.perf_counter()
    res = run_bass_kernel_spmd(nc, in_maps, core_ids=list(range(NCORES)))
    _cache["last_run_wall_s"] = _time.perf_counter() - _t0

    out = np.zeros((N_GRAPHS, N_CLASSES), np.float32)
    for k in range(NCORES):
        ng = gstart[k + 1] - gstart[k]
        out[gstart[k]:gstart[k + 1]] = res.results[k]["logitsT"][:, :ng].T
    return out



# revision 8
# speedup vs baseline: 15.2174x; 15.2174x over previous
"""DGL-life GCN classifier on 8 Trainium2 NeuronCores (Bass/Tile).

Strategy: shard the batched graph by dst-node across 8 cores (cuts aligned to
graph boundaries).  The axon tunnel to the devices moves ~35-40 MB/s, so the
wire format is minimized: each core receives only its local bf16 feature
shard (the full table is built on-device with an AllGather), edge metadata is
packed one int32 word per edge slot (gather-row index | one-hot lane << 19),
and all weights ride in a single fp32 blob.  Per layer: per-edge rows are
fetched with indirect DMA gathers, segment-summed into 128-dst blocks via
one-hot matmuls accumulated in PSUM, then the dense GraphConv / residual
transforms run in the transposed domain where the per-output-feature bias
rides the ACT relu for free.  Feature shards are exchanged between layers
with an AllGather.  SumPooling reuses the same one-hot matmul machinery over
graph ids, followed by the 2-layer MLP classifier.
"""
import sys
sys.path.insert(0, "/opt/trn_rl_repo")

import numpy as np
import ml_dtypes

bf16 = ml_dtypes.bfloat16

N_NODES = 500000
N_EDGES = 8000000
N_GRAPHS = 16384
IN_F = 74
HID = 64
CLS_H = 128
N_CLASSES = 2

NCORES = 8
NPAD = 62976          # padded nodes per shard (492 blocks of 128)
NBLK = 492
GRP = 4               # dst blocks per For_i group
NGRP = NBLK // GRP    # 123
EBLK = 2304           # edge-slot capacity per dst block (18 tiles of 128)
TPB = EBLK // 128     # 18
COLS = GRP * TPB      # 72 gather columns per group
NTOT = NCORES * NPAD  # 503808 padded global rows
GPAD = 2176           # padded graphs per shard (17 blocks of 128)
NGB = GPAD // 128     # 17
PT = 33               # pooling node tiles per graph block
ETOT = NGRP * COLS    # edge-meta columns
PTOT = NGB * PT       # pool-meta columns

# weight blob layout: (name, rows, cols); first 8 are bf16 matmul weights
WSPECS = [
    ("gW0", IN_F, HID), ("rW0", IN_F, HID),
    ("gW1", HID, HID), ("rW1", HID, HID),
    ("gW2", HID, HID), ("rW2", HID, HID),
    ("cW1", HID, CLS_H), ("cW2", CLS_H, N_CLASSES),
    ("gb0", HID, 1), ("rb0", HID, 1), ("gb1", HID, 1), ("rb1", HID, 1),
    ("gb2", HID, 1), ("rb2", HID, 1), ("cb1", CLS_H, 1), ("cb2", N_CLASSES, 1),
]
WTOT = sum(k * m for _, k, m in WSPECS)

_cache = {}


def _host_prep(node_feats, src, dst, graph_ids):
    gid = np.asarray(graph_ids)
    node_of_graph_start = np.searchsorted(gid, np.arange(N_GRAPHS))
    cuts = [0]
    for k in range(1, NCORES):
        target = k * N_NODES // NCORES
        gi = np.searchsorted(node_of_graph_start, target)
        cand = [node_of_graph_start[min(gi, N_GRAPHS - 1)],
                node_of_graph_start[max(gi - 1, 0)]]
        cuts.append(int(min(cand, key=lambda x: abs(x - target))))
    cuts.append(N_NODES)
    cuts = np.asarray(cuts, np.int64)
    shard_sizes = np.diff(cuts)
    assert shard_sizes.max() <= NPAD

    gstart = [int(gid[c]) if c < N_NODES else N_GRAPHS for c in cuts[:-1]] + [N_GRAPHS]
    gstart = np.asarray(gstart, np.int64)
    assert np.diff(gstart).max() <= GPAD

    src = np.asarray(src).astype(np.int64)
    dst = np.asarray(dst).astype(np.int64)
    shard_of_src = np.searchsorted(cuts, src, side="right") - 1
    src_pad = shard_of_src * NPAD + (src - cuts[shard_of_src])

    per_core = []
    shard_of_dst = np.searchsorted(cuts, dst, side="right") - 1
    for k in range(NCORES):
        base, n = cuts[k], cuts[k + 1] - cuts[k]
        m = shard_of_dst == k
        e_src = src_pad[m]
        e_dst = dst[m] - base
        order = np.argsort(e_dst, kind="stable")
        e_src, e_dst = e_src[order], e_dst[order]
        blk = e_dst >> 7
        cnt = np.bincount(blk, minlength=NBLK)
        assert cnt.max() <= EBLK, cnt.max()
        cum = np.concatenate([[0], np.cumsum(cnt)])
        slot = np.arange(len(e_dst)) - cum[blk]
        # packed word: gather row index (19 bits) | one-hot lane (255=invalid)
        eword = np.full((NBLK, EBLK), 255 << 19, np.int32)
        eword[blk, slot] = (e_src | ((e_dst & 127) << 19)).astype(np.int32)
        eword = (eword.reshape(NGRP, GRP, TPB, 128)
                 .transpose(3, 0, 1, 2).reshape(128, ETOT))

        # pooling: local nodes sorted by graph; graph-block-aligned slots
        gl = gid[base:base + n] - gstart[k]          # local graph id per node
        gb = gl >> 7
        pcnt = np.bincount(gb, minlength=NGB)
        assert pcnt.max() <= PT * 128
        pcum = np.concatenate([[0], np.cumsum(pcnt)])
        pslot = np.arange(n) - pcum[gb]
        pword = np.full((NGB, PT * 128), 255 << 16, np.int32)
        pword[gb, pslot] = (np.arange(n, dtype=np.int64)
                            | ((gl & 127) << 16)).astype(np.int32)
        pword = (pword.reshape(NGB, PT, 128)
                 .transpose(2, 0, 1).reshape(128, PTOT))

        meta = np.concatenate([eword, pword], axis=1)

        h0 = np.zeros((NPAD, IN_F), bf16)
        h0[:n] = node_feats[base:base + n].astype(bf16)

        per_core.append(dict(meta=np.ascontiguousarray(meta), h0=h0))
    return cuts, gstart, per_core


def _build_nc():
    import concourse.bass as bass
    from concourse import bacc
    import concourse.mybir as mybir
    import concourse.tile as tile

    fp32 = mybir.dt.float32
    b16 = mybir.dt.bfloat16
    i32 = mybir.dt.int32

    nc = bacc.Bacc("TRN2", target_bir_lowering=False, debug=False,
                   num_devices=NCORES)

    h0_in = nc.dram_tensor("h0", [NPAD, IN_F], b16, kind="ExternalInput")
    meta_in = nc.dram_tensor("meta", [128, ETOT + PTOT], i32, kind="ExternalInput")
    wblob_in = nc.dram_tensor("wblob", [WTOT], fp32, kind="ExternalInput")
    logits_out = nc.dram_tensor("logitsT", [N_CLASSES, GPAD], fp32, kind="ExternalOutput")

    emeta_ap = meta_in[:, 0:ETOT].rearrange("p (g c) -> p g c", c=COLS)
    pmeta_ap = meta_in[:, ETOT:ETOT + PTOT].rearrange("p (g c) -> p g c", c=PT)
    # L0 residual rhs source: [g, b, 128, f] view of the local shard
    h0r_ap = h0_in[:].rearrange("(g b p) f -> g b p f", b=GRP, p=128)

    Relu = mybir.ActivationFunctionType.Relu
    EQ = mybir.AluOpType.is_equal
    AND = mybir.AluOpType.bitwise_and
    SHR = mybir.AluOpType.logical_shift_right
    ADD = mybir.AluOpType.add

    with tile.TileContext(nc) as tc:
        with (
            tc.tile_pool(name="const", bufs=1) as constp,
            tc.tile_pool(name="persist", bufs=1) as persistp,
            tc.tile_pool(name="meta", bufs=2) as metap,
            tc.tile_pool(name="slab", bufs=2) as slabp,
            tc.tile_pool(name="p1", bufs=4) as pp,
            tc.tile_pool(name="sb", bufs=4) as sbp,
            tc.tile_pool(name="agg_ps", bufs=2, space="PSUM") as aggps,
            tc.tile_pool(name="mm_ps", bufs=2, space="PSUM") as mmps,
            tc.tile_pool(name="tp_ps", bufs=2, space="PSUM") as tpps,
            tc.tile_pool(name="dram", bufs=1, space="DRAM") as dramp,
        ):
            # on-device constants: iota row + bf16 identity
            iota_i = constp.tile([128, 128], i32)
            nc.gpsimd.iota(iota_i[:], pattern=[[1, 128]], base=0,
                           channel_multiplier=0)
            ones = constp.tile([128, 128], b16)
            nc.vector.memset(ones[:], 1.0)
            ident = constp.tile([128, 128], b16)
            nc.gpsimd.affine_select(out=ident[:], in_=ones[:],
                                    pattern=[[-1, 128]], compare_op=EQ,
                                    fill=0.0, base=0, channel_multiplier=1)

            # weights from the blob: matmul weights cast to bf16 via SWDGE,
            # biases stay fp32
            wt = {}
            off = 0
            for name, k, m in WSPECS:
                ap = wblob_in[off:off + k * m].rearrange("(k m) -> k m", m=m)
                if m > 1:
                    t = constp.tile([k, m], b16, tag=f"w16_{name}",
                                    name=f"w16_{name}")
                    nc.gpsimd.dma_start(t[:], ap)
                else:
                    t = constp.tile([k, 1], fp32, tag=f"b_{name}",
                                    name=f"b_{name}")
                    nc.sync.dma_start(t[:], ap)
                wt[name] = t
                off += k * m
            gW = [wt["gW0"], wt["gW1"], wt["gW2"]]
            rW = [wt["rW0"], wt["rW1"], wt["rW2"]]
            gb = [wt["gb0"], wt["gb1"], wt["gb2"]]
            rb = [wt["rb0"], wt["rb1"], wt["rb2"]]
            cW1, cW2, cb1, cb2 = wt["cW1"], wt["cW2"], wt["cb1"], wt["cb2"]

            # DRAM scratch: full bf16 tables (AllGathered), local transposed
            # features, final local features
            table0 = dramp.tile([NTOT, IN_F], b16, addr_space="Shared",
                                name="table0")
            cc_in = [dramp.tile([NPAD, HID], b16, name=f"cc_in{i}") for i in range(2)]
            cc_out = [dramp.tile([NTOT, HID], b16, addr_space="Shared",
                                 name=f"cc_out{i}") for i in range(2)]
            hT_dram = dramp.tile([HID, NPAD], b16, name="hT_dram")
            h3_local = dramp.tile([NPAD, HID], b16, name="h3_local")

            h0_stage = dramp.tile([NPAD, IN_F], b16, name="h0_stage")
            nc.sync.dma_start(h0_stage[:], h0_in[:])
            nc.gpsimd.collective_compute(
                "AllGather", mybir.AluOpType.bypass,
                replica_groups=[list(range(NCORES))],
                ins=[h0_stage[:].opt()], outs=[table0[:].opt()])

            for L in range(3):
                DIN = IN_F if L == 0 else HID
                table_ap = table0[:] if L == 0 else cc_out[L - 1][:]
                with tc.For_i(0, NGRP, 1) as g:
                    w_t = metap.tile([128, COLS], i32, tag="w")
                    nc.sync.dma_start(w_t[:], emeta_ap[:, bass.ds(g, 1), :])
                    idx_t = metap.tile([128, COLS], i32, tag="idx")
                    nc.vector.tensor_scalar(out=idx_t[:], in0=w_t[:],
                                            scalar1=0x7FFFF, scalar2=None,
                                            op0=AND)
                    lane_t = metap.tile([128, COLS], i32, tag="lane")
                    nc.vector.tensor_scalar(out=lane_t[:], in0=w_t[:],
                                            scalar1=19, scalar2=None, op0=SHR)

                    slab = slabp.tile([128, COLS, IN_F], b16, tag="slab",
                                      name="slab")[:, :, :DIN]
                    for j in range(COLS):
                        nc.gpsimd.indirect_dma_start(
                            out=slab[:, j, :], out_offset=None, in_=table_ap,
                            in_offset=bass.IndirectOffsetOnAxis(
                                ap=idx_t[:, j:j + 1], axis=0))

                    # residual rhs: transposed local features for this group
                    hTg = slabp.tile([IN_F, GRP * 128], b16, tag="hTg",
                                     name="hTg")[:DIN, :]
                    if L == 0:
                        h0blk = slabp.tile([128, GRP * IN_F], b16, tag="h0blk")
                        for b6 in range(GRP):
                            nc.sync.dma_start(
                                h0blk[:, b6 * IN_F:(b6 + 1) * IN_F],
                                h0r_ap[bass.ds(g, 1), b6:b6 + 1, :, :])
                        for b6 in range(GRP):
                            tp0 = tpps.tile([IN_F, 128], b16, space="PSUM",
                                            tag="tp")
                            nc.tensor.transpose(
                                out=tp0[:], in_=h0blk[:, b6 * IN_F:(b6 + 1) * IN_F],
                                identity=ident[:])
                            nc.vector.tensor_copy(
                                hTg[:, b6 * 128:(b6 + 1) * 128], tp0[:])
                    else:
                        nc.sync.dma_start(hTg[:], hT_dram[:, bass.ts(g, GRP * 128)])

                    for b6 in range(GRP):
                        psum = aggps.tile([DIN, 128], fp32, space="PSUM", tag="agg")
                        for t in range(TPB):
                            j = b6 * TPB + t
                            p_t = pp.tile([128, 128], b16)
                            nc.vector.tensor_tensor(
                                out=p_t[:],
                                in0=lane_t[:, j:j + 1].to_broadcast([128, 128]),
                                in1=iota_i[:], op=EQ)
                            nc.tensor.matmul(out=psum[:], lhsT=slab[:, j, :],
                                             rhs=p_t[:], start=(t == 0),
                                             stop=(t == TPB - 1))
                        aggT = sbp.tile([DIN, 128], b16, tag="aggT")
                        nc.vector.tensor_copy(aggT[:], psum[:])
                        convp = mmps.tile([HID, 128], fp32, space="PSUM", tag="conv")
                        nc.tensor.matmul(out=convp[:], lhsT=gW[L][:], rhs=aggT[:],
                                         start=True, stop=True)
                        hTb = hTg[:, b6 * 128:(b6 + 1) * 128]
                        resp = mmps.tile([HID, 128], fp32, space="PSUM", tag="res")
                        nc.tensor.matmul(out=resp[:], lhsT=rW[L][:], rhs=hTb,
                                         start=True, stop=True)
                        convs = sbp.tile([HID, 128], b16, tag="convs")
                        nc.scalar.activation(convs[:], convp[:], Relu, bias=gb[L][:, :1])
                        ress = sbp.tile([HID, 128], b16, tag="ress")
                        nc.scalar.activation(ress[:], resp[:], Relu, bias=rb[L][:, :1])
                        hnewT = sbp.tile([HID, 128], b16, tag="hnewT")
                        nc.vector.tensor_add(hnewT[:], convs[:], ress[:])
                        if L < 2:
                            nc.sync.dma_start(
                                hT_dram[:, bass.ts(g, GRP * 128)][:, b6 * 128:(b6 + 1) * 128],
                                hnewT[:])
                        tp = tpps.tile([128, HID], b16, space="PSUM", tag="tp")
                        nc.tensor.transpose(out=tp[:], in_=hnewT[:],
                                            identity=ident[:HID, :HID])
                        hnew = sbp.tile([128, HID], b16, tag="hnew")
                        nc.vector.tensor_copy(hnew[:], tp[:])
                        dst_dram = cc_in[L] if L < 2 else h3_local
                        dd = dst_dram[:].rearrange("(g x) d -> g x d", x=GRP * 128)
                        nc.sync.dma_start(
                            dd[bass.ds(g, 1), b6 * 128:(b6 + 1) * 128, :], hnew[:])
                if L < 2:
                    nc.gpsimd.collective_compute(
                        "AllGather", mybir.AluOpType.bypass,
                        replica_groups=[list(range(NCORES))],
                        ins=[cc_in[L][:].opt()], outs=[cc_out[L][:].opt()])

            # -------- pooling + classifier --------
            out_slab = persistp.tile([N_CLASSES, GPAD], fp32)
            with tc.For_i(0, NGB, 1) as gbv:
                wp_t = metap.tile([128, PT], i32, tag="wp")
                nc.sync.dma_start(wp_t[:], pmeta_ap[:, bass.ds(gbv, 1), :])
                pidx_t = metap.tile([128, PT], i32, tag="pidx")
                nc.vector.tensor_scalar(out=pidx_t[:], in0=wp_t[:],
                                        scalar1=0xFFFF, scalar2=None, op0=AND)
                plane_t = metap.tile([128, PT], i32, tag="plane")
                nc.vector.tensor_scalar(out=plane_t[:], in0=wp_t[:],
                                        scalar1=16, scalar2=None, op0=SHR)
                pslab = slabp.tile([128, PT, HID], b16, tag="pslab")
                for t in range(PT):
                    nc.gpsimd.indirect_dma_start(
                        out=pslab[:, t, :], out_offset=None, in_=h3_local[:],
                        in_offset=bass.IndirectOffsetOnAxis(
                            ap=pidx_t[:, t:t + 1], axis=0))
                gpsum = aggps.tile([HID, 128], fp32, space="PSUM", tag="agg",
                                   name="gpsum")
                for t in range(PT):
                    p_t = pp.tile([128, 128], b16, tag="pp")
                    nc.vector.tensor_tensor(
                        out=p_t[:], in0=plane_t[:, t:t + 1].to_broadcast([128, 128]),
                        in1=iota_i[:], op=EQ)
                    nc.tensor.matmul(out=gpsum[:], lhsT=pslab[:, t, :], rhs=p_t[:],
                                     start=(t == 0), stop=(t == PT - 1))
                graphT = sbp.tile([HID, 128], b16, tag="graphT")
                nc.vector.tensor_copy(graphT[:], gpsum[:])
                hidp = mmps.tile([CLS_H, 128], fp32, space="PSUM", tag="conv",
                                 name="hidp")
                nc.tensor.matmul(out=hidp[:], lhsT=cW1[:], rhs=graphT[:],
                                 start=True, stop=True)
                hid = sbp.tile([CLS_H, 128], b16, tag="hids")
                nc.scalar.activation(hid[:], hidp[:], Relu, bias=cb1[:, :1])
                logp = tpps.tile([N_CLASSES, 128], fp32, space="PSUM", tag="tp",
                                 name="logp")
                nc.tensor.matmul(out=logp[:], lhsT=cW2[:], rhs=hid[:],
                                 start=True, stop=True)
                nc.vector.tensor_tensor(
                    out=out_slab[:, bass.ts(gbv, 128)],
                    in0=cb2[:, 0:1].to_broadcast([N_CLASSES, 128]),
                    in1=logp[:], op=ADD)
            nc.sync.dma_start(logits_out[:], out_slab[:])

    nc.compile()
    return nc


def kernel(node_feats, src, dst, graph_ids,
           gW0, gb0, rW0, rb0, gW1, gb1, rW1, rb1, gW2, gb2, rW2, rb2,
           cW1, cb1, cW2, cb2):
    from concourse.bass_utils import run_bass_kernel_spmd

    node_feats = np.asarray(node_feats)
    cuts, gstart, per_core = _host_prep(node_feats, src, dst, graph_ids)

    if "nc" not in _cache:
        _cache["nc"] = _build_nc()
    nc = _cache["nc"]

    wvals = dict(
        gW0=gW0, rW0=rW0, gW1=gW1, rW1=rW1, gW2=gW2, rW2=rW2,
        cW1=cW1, cW2=cW2, gb0=gb0, rb0=rb0, gb1=gb1, rb1=rb1,
        gb2=gb2, rb2=rb2, cb1=cb1, cb2=cb2,
    )
    wblob = np.concatenate(
        [np.asarray(wvals[name], np.float32).reshape(-1) for name, _, _ in WSPECS])

    in_maps = [dict(h0=per_core[k]["h0"], meta=per_core[k]["meta"], wblob=wblob)
               for k in range(NCORES)]

    import time as _time
    _t0 = _time.perf_counter()
    res = run_bass_kernel_spmd(nc, in_maps, core_ids=list(range(NCORES)))
    _cache["last_run_wall_s"] = _time.perf_counter() - _t0

    out = np.zeros((N_GRAPHS, N_CLASSES), np.float32)
    for k in range(NCORES):
        ng = gstart[k + 1] - gstart[k]
        out[gstart[k]:gstart[k + 1]] = res.results[k]["logitsT"][:, :ng].T
    return out


# revision 14
# speedup vs baseline: 19.4377x; 1.2773x over previous
"""DGL-life GCN classifier on 8 Trainium2 NeuronCores (Bass/Tile).

Strategy: shard the batched graph by dst-node across 8 cores (cuts aligned to
graph boundaries).  The axon tunnel to the devices moves ~35-40 MB/s, so the
wire format is minimized: each core receives only its local bf16 feature
shard (the full table is built on-device with an AllGather), edge metadata is
packed one int32 word per edge slot (gather-row index | one-hot lane << 19),
and all weights ride in a single fp32 blob.  Per layer: per-edge rows are
fetched with indirect DMA gathers, segment-summed into 128-dst blocks via
one-hot matmuls accumulated in PSUM, then the dense GraphConv / residual
transforms run in the transposed domain where the per-output-feature bias
rides the ACT relu for free.  Feature shards are exchanged between layers
with an AllGather.  SumPooling reuses the same one-hot matmul machinery over
graph ids, followed by the 2-layer MLP classifier.
"""
import sys
sys.path.insert(0, "/opt/trn_rl_repo")

import numpy as np
import ml_dtypes

bf16 = ml_dtypes.bfloat16
fp8 = ml_dtypes.float8_e4m3

N_NODES = 500000
N_EDGES = 8000000
N_GRAPHS = 16384
IN_F = 74
HID = 64
CLS_H = 128
N_CLASSES = 2

NCORES = 8
NPAD = 62976          # padded nodes per shard (492 blocks of 128)
NBLK = 492
GRP = 4               # dst blocks per For_i group
NGRP = NBLK // GRP    # 123
EBLK = 2304           # edge-slot capacity per dst block (18 tiles of 128)
TPB = EBLK // 128     # 18
COLS = GRP * TPB      # 72 gather columns per group
NTOT = NCORES * NPAD  # 503808 padded global rows
GPAD = 2176           # padded graphs per shard (17 blocks of 128)
NGB = GPAD // 128     # 17
PT = 33               # pooling node tiles per graph block
ETOT = NGRP * COLS    # edge-meta columns
PTOT = NGB * PT       # pool-meta columns

# weight blob layout: (name, rows, cols); first 8 are bf16 matmul weights
WSPECS = [
    ("gW0", IN_F, HID), ("rW0", IN_F, HID),
    ("gW1", HID, HID), ("rW1", HID, HID),
    ("gW2", HID, HID), ("rW2", HID, HID),
    ("cW1", HID, CLS_H), ("cW2", CLS_H, N_CLASSES),
    ("gb0", HID, 1), ("rb0", HID, 1), ("gb1", HID, 1), ("rb1", HID, 1),
    ("gb2", HID, 1), ("rb2", HID, 1), ("cb1", CLS_H, 1), ("cb2", N_CLASSES, 1),
]
WTOT = sum(k * m for _, k, m in WSPECS)

_cache = {}


def _host_prep(node_feats, src, dst, graph_ids):
    gid = np.asarray(graph_ids)
    node_of_graph_start = np.searchsorted(gid, np.arange(N_GRAPHS))
    cuts = [0]
    for k in range(1, NCORES):
        target = k * N_NODES // NCORES
        gi = np.searchsorted(node_of_graph_start, target)
        cand = [node_of_graph_start[min(gi, N_GRAPHS - 1)],
                node_of_graph_start[max(gi - 1, 0)]]
        cuts.append(int(min(cand, key=lambda x: abs(x - target))))
    cuts.append(N_NODES)
    cuts = np.asarray(cuts, np.int64)
    shard_sizes = np.diff(cuts)
    assert shard_sizes.max() <= NPAD

    gstart = [int(gid[c]) if c < N_NODES else N_GRAPHS for c in cuts[:-1]] + [N_GRAPHS]
    gstart = np.asarray(gstart, np.int64)
    assert np.diff(gstart).max() <= GPAD

    src = np.asarray(src).astype(np.int64)
    dst = np.asarray(dst).astype(np.int64)
    shard_of_src = np.searchsorted(cuts, src, side="right") - 1
    src_pad = shard_of_src * NPAD + (src - cuts[shard_of_src])

    per_core = []
    shard_of_dst = np.searchsorted(cuts, dst, side="right") - 1
    for k in range(NCORES):
        base, n = cuts[k], cuts[k + 1] - cuts[k]
        m = shard_of_dst == k
        e_src = src_pad[m]
        e_dst = dst[m] - base
        order = np.argsort(e_dst, kind="stable")
        e_src, e_dst = e_src[order], e_dst[order]
        blk = e_dst >> 7
        cnt = np.bincount(blk, minlength=NBLK)
        assert cnt.max() <= EBLK, cnt.max()
        cum = np.concatenate([[0], np.cumsum(cnt)])
        slot = np.arange(len(e_dst)) - cum[blk]
        # packed word: gather row index (19 bits) | one-hot lane (255=invalid)
        eword = np.full((NBLK, EBLK), 255 << 19, np.int32)
        eword[blk, slot] = (e_src | ((e_dst & 127) << 19)).astype(np.int32)
        eword = (eword.reshape(NGRP, GRP, TPB, 128)
                 .transpose(3, 0, 1, 2).reshape(128, ETOT))

        # pooling: local nodes sorted by graph; graph-block-aligned slots
        gl = gid[base:base + n] - gstart[k]          # local graph id per node
        gb = gl >> 7
        pcnt = np.bincount(gb, minlength=NGB)
        assert pcnt.max() <= PT * 128
        pcum = np.concatenate([[0], np.cumsum(pcnt)])
        pslot = np.arange(n) - pcum[gb]
        pword = np.full((NGB, PT * 128), 255 << 16, np.int32)
        pword[gb, pslot] = (np.arange(n, dtype=np.int64)
                            | ((gl & 127) << 16)).astype(np.int32)
        pword = (pword.reshape(NGB, PT, 128)
                 .transpose(2, 0, 1).reshape(128, PTOT))

        meta = np.concatenate([eword, pword], axis=1)

        h0 = np.zeros((NPAD, IN_F), fp8)
        h0[:n] = node_feats[base:base + n].astype(fp8)

        per_core.append(dict(meta=np.ascontiguousarray(meta), h0=h0))
    return cuts, gstart, per_core


def _build_nc():
    import concourse.bass as bass
    from concourse import bacc
    import concourse.mybir as mybir
    import concourse.tile as tile

    fp32 = mybir.dt.float32
    b16 = mybir.dt.bfloat16
    f8 = mybir.dt.float8e4
    i32 = mybir.dt.int32

    nc = bacc.Bacc("TRN2", target_bir_lowering=False, debug=False,
                   num_devices=NCORES)

    h0_in = nc.dram_tensor("h0", [NPAD, IN_F], f8, kind="ExternalInput")
    meta_in = nc.dram_tensor("meta", [128, ETOT + PTOT], i32, kind="ExternalInput")
    wblob_in = nc.dram_tensor("wblob", [WTOT], fp32, kind="ExternalInput")
    logits_out = nc.dram_tensor("logitsT", [N_CLASSES, GPAD], fp32, kind="ExternalOutput")

    emeta_ap = meta_in[:, 0:ETOT].rearrange("p (g c) -> p g c", c=COLS)
    pmeta_ap = meta_in[:, ETOT:ETOT + PTOT].rearrange("p (g c) -> p g c", c=PT)
    # L0 residual rhs source: [g, b, 128, f] view of the local shard
    h0r_ap = h0_in[:].rearrange("(g b p) f -> g b p f", b=GRP, p=128)

    Relu = mybir.ActivationFunctionType.Relu
    EQ = mybir.AluOpType.is_equal
    AND = mybir.AluOpType.bitwise_and
    SHR = mybir.AluOpType.logical_shift_right
    ADD = mybir.AluOpType.add

    with tile.TileContext(nc) as tc:
        with (
            tc.tile_pool(name="const", bufs=1) as constp,
            tc.tile_pool(name="persist", bufs=1) as persistp,
            tc.tile_pool(name="meta", bufs=2) as metap,
            tc.tile_pool(name="slab", bufs=2) as slabp,
            tc.tile_pool(name="p1", bufs=4) as pp,
            tc.tile_pool(name="sb", bufs=4) as sbp,
            tc.tile_pool(name="agg_ps", bufs=2, space="PSUM") as aggps,
            tc.tile_pool(name="mm_ps", bufs=2, space="PSUM") as mmps,
            tc.tile_pool(name="tp_ps", bufs=2, space="PSUM") as tpps,
            tc.tile_pool(name="dram", bufs=1, space="DRAM") as dramp,
        ):
            # on-device constants: iota row + bf16 identity
            iota_i = constp.tile([128, 128], i32)
            nc.gpsimd.iota(iota_i[:], pattern=[[1, 128]], base=0,
                           channel_multiplier=0)
            ones = constp.tile([128, 128], b16)
            nc.vector.memset(ones[:], 1.0)
            ident = constp.tile([128, 128], b16)
            nc.gpsimd.affine_select(out=ident[:], in_=ones[:],
                                    pattern=[[-1, 128]], compare_op=EQ,
                                    fill=0.0, base=0, channel_multiplier=1)

            # weights from the blob: matmul weights cast to bf16 via SWDGE,
            # biases stay fp32
            wt = {}
            off = 0
            for name, k, m in WSPECS:
                ap = wblob_in[off:off + k * m].rearrange("(k m) -> k m", m=m)
                if m > 1:
                    t = constp.tile([k, m], b16, tag=f"w16_{name}",
                                    name=f"w16_{name}")
                    nc.gpsimd.dma_start(t[:], ap)
                else:
                    t = constp.tile([k, 1], fp32, tag=f"b_{name}",
                                    name=f"b_{name}")
                    nc.sync.dma_start(t[:], ap)
                wt[name] = t
                off += k * m
            gW = [wt["gW0"], wt["gW1"], wt["gW2"]]
            rW = [wt["rW0"], wt["rW1"], wt["rW2"]]
            gb = [wt["gb0"], wt["gb1"], wt["gb2"]]
            rb = [wt["rb0"], wt["rb1"], wt["rb2"]]
            cW1, cW2, cb1, cb2 = wt["cW1"], wt["cW2"], wt["cb1"], wt["cb2"]

            # DRAM scratch: full bf16 tables (AllGathered), local transposed
            # features, final local features
            table0 = dramp.tile([NTOT, IN_F], f8, addr_space="Shared",
                                name="table0")
            cc_in = [dramp.tile([NPAD, HID], b16, name=f"cc_in{i}") for i in range(2)]
            cc_out = [dramp.tile([NTOT, HID], b16, addr_space="Shared",
                                 name=f"cc_out{i}") for i in range(2)]
            hT_dram = dramp.tile([HID, NPAD], b16, name="hT_dram")
            h3_local = dramp.tile([NPAD, HID], b16, name="h3_local")

            h0_stage = dramp.tile([NPAD, IN_F], f8, name="h0_stage")
            nc.sync.dma_start(h0_stage[:], h0_in[:])
            nc.gpsimd.collective_compute(
                "AllGather", mybir.AluOpType.bypass,
                replica_groups=[list(range(NCORES))],
                ins=[h0_stage[:].opt()], outs=[table0[:].opt()])

            for L in range(3):
                DIN = IN_F if L == 0 else HID
                table_ap = table0[:] if L == 0 else cc_out[L - 1][:]
                with tc.For_i(0, NGRP, 1) as g:
                    w_t = metap.tile([128, COLS], i32, tag="w")
                    nc.sync.dma_start(w_t[:], emeta_ap[:, bass.ds(g, 1), :])
                    idx_t = metap.tile([128, COLS], i32, tag="idx")
                    nc.vector.tensor_scalar(out=idx_t[:], in0=w_t[:],
                                            scalar1=0x7FFFF, scalar2=None,
                                            op0=AND)
                    lane_t = metap.tile([128, COLS], i32, tag="lane")
                    nc.vector.tensor_scalar(out=lane_t[:], in0=w_t[:],
                                            scalar1=19, scalar2=None, op0=SHR)

                    if L == 0:
                        slab = slabp.tile([128, COLS, IN_F], f8, tag="slab8",
                                          name="slab8")[:, :, :DIN]
                    else:
                        slab = slabp.tile([128, COLS, IN_F], b16, tag="slab",
                                          name="slab")[:, :, :DIN]
                    for j in range(COLS):
                        nc.gpsimd.indirect_dma_start(
                            out=slab[:, j, :], out_offset=None, in_=table_ap,
                            in_offset=bass.IndirectOffsetOnAxis(
                                ap=idx_t[:, j:j + 1], axis=0))

                    # residual rhs: transposed local features for this group
                    hTg = slabp.tile([IN_F, GRP * 128], b16, tag="hTg",
                                     name="hTg")[:DIN, :]
                    if L == 0:
                        h0blk8 = slabp.tile([128, GRP * IN_F], f8, tag="h0blk8")
                        for b6 in range(GRP):
                            nc.sync.dma_start(
                                h0blk8[:, b6 * IN_F:(b6 + 1) * IN_F],
                                h0r_ap[bass.ds(g, 1), b6:b6 + 1, :, :])
                        h0blk = slabp.tile([128, GRP * IN_F], b16, tag="h0blk")
                        nc.vector.tensor_copy(h0blk[:], h0blk8[:])
                        for b6 in range(GRP):
                            tp0 = tpps.tile([IN_F, 128], b16, space="PSUM",
                                            tag="tp")
                            nc.tensor.transpose(
                                out=tp0[:], in_=h0blk[:, b6 * IN_F:(b6 + 1) * IN_F],
                                identity=ident[:])
                            nc.vector.tensor_copy(
                                hTg[:, b6 * 128:(b6 + 1) * 128], tp0[:])
                    else:
                        nc.sync.dma_start(hTg[:], hT_dram[:, bass.ts(g, GRP * 128)])

                    for b6 in range(GRP):
                        psum = aggps.tile([DIN, 128], fp32, space="PSUM", tag="agg")
                        for t in range(TPB):
                            j = b6 * TPB + t
                            if L == 0:
                                p_t = pp.tile([128, 128], f8, tag="p8",
                                              name="p8")
                            else:
                                p_t = pp.tile([128, 128], b16, tag="p16",
                                              name="p16")
                            nc.vector.tensor_tensor(
                                out=p_t[:],
                                in0=lane_t[:, j:j + 1].to_broadcast([128, 128]),
                                in1=iota_i[:], op=EQ)
                            nc.tensor.matmul(out=psum[:], lhsT=slab[:, j, :],
                                             rhs=p_t[:], start=(t == 0),
                                             stop=(t == TPB - 1))
                        aggT = sbp.tile([DIN, 128], b16, tag="aggT")
                        nc.vector.tensor_copy(aggT[:], psum[:])
                        convp = mmps.tile([HID, 128], fp32, space="PSUM", tag="conv")
                        nc.tensor.matmul(out=convp[:], lhsT=gW[L][:], rhs=aggT[:],
                                         start=True, stop=True)
                        hTb = hTg[:, b6 * 128:(b6 + 1) * 128]
                        resp = mmps.tile([HID, 128], fp32, space="PSUM", tag="res")
                        nc.tensor.matmul(out=resp[:], lhsT=rW[L][:], rhs=hTb,
                                         start=True, stop=True)
                        convs = sbp.tile([HID, 128], b16, tag="convs")
                        nc.scalar.activation(convs[:], convp[:], Relu, bias=gb[L][:, :1])
                        ress = sbp.tile([HID, 128], b16, tag="ress")
                        nc.scalar.activation(ress[:], resp[:], Relu, bias=rb[L][:, :1])
                        hnewT = sbp.tile([HID, 128], b16, tag="hnewT")
                        nc.vector.tensor_add(hnewT[:], convs[:], ress[:])
                        if L < 2:
                            nc.sync.dma_start(
                                hT_dram[:, bass.ts(g, GRP * 128)][:, b6 * 128:(b6 + 1) * 128],
                                hnewT[:])
                        tp = tpps.tile([128, HID], b16, space="PSUM", tag="tp")
                        nc.tensor.transpose(out=tp[:], in_=hnewT[:],
                                            identity=ident[:HID, :HID])
                        hnew = sbp.tile([128, HID], b16, tag="hnew")
                        nc.vector.tensor_copy(hnew[:], tp[:])
                        dst_dram = cc_in[L] if L < 2 else h3_local
                        dd = dst_dram[:].rearrange("(g x) d -> g x d", x=GRP * 128)
                        nc.sync.dma_start(
                            dd[bass.ds(g, 1), b6 * 128:(b6 + 1) * 128, :], hnew[:])
                if L < 2:
                    nc.gpsimd.collective_compute(
                        "AllGather", mybir.AluOpType.bypass,
                        replica_groups=[list(range(NCORES))],
                        ins=[cc_in[L][:].opt()], outs=[cc_out[L][:].opt()])

            # -------- pooling + classifier --------
            out_slab = persistp.tile([N_CLASSES, GPAD], fp32)
            with tc.For_i(0, NGB, 1) as gbv:
                wp_t = metap.tile([128, PT], i32, tag="wp")
                nc.sync.dma_start(wp_t[:], pmeta_ap[:, bass.ds(gbv, 1), :])
                pidx_t = metap.tile([128, PT], i32, tag="pidx")
                nc.vector.tensor_scalar(out=pidx_t[:], in0=wp_t[:],
                                        scalar1=0xFFFF, scalar2=None, op0=AND)
                plane_t = metap.tile([128, PT], i32, tag="plane")
                nc.vector.tensor_scalar(out=plane_t[:], in0=wp_t[:],
                                        scalar1=16, scalar2=None, op0=SHR)
                pslab = slabp.tile([128, PT, HID], b16, tag="pslab")
                for t in range(PT):
                    nc.gpsimd.indirect_dma_start(
                        out=pslab[:, t, :], out_offset=None, in_=h3_local[:],
                        in_offset=bass.IndirectOffsetOnAxis(
                            ap=pidx_t[:, t:t + 1], axis=0))
                gpsum = aggps.tile([HID, 128], fp32, space="PSUM", tag="agg",
                                   name="gpsum")
                for t in range(PT):
                    p_t = pp.tile([128, 128], b16, tag="pp")
                    nc.vector.tensor_tensor(
                        out=p_t[:], in0=plane_t[:, t:t + 1].to_broadcast([128, 128]),
                        in1=iota_i[:], op=EQ)
                    nc.tensor.matmul(out=gpsum[:], lhsT=pslab[:, t, :], rhs=p_t[:],
                                     start=(t == 0), stop=(t == PT - 1))
                graphT = sbp.tile([HID, 128], b16, tag="graphT")
                nc.vector.tensor_copy(graphT[:], gpsum[:])
                hidp = mmps.tile([CLS_H, 128], fp32, space="PSUM", tag="conv",
                                 name="hidp")
                nc.tensor.matmul(out=hidp[:], lhsT=cW1[:], rhs=graphT[:],
                                 start=True, stop=True)
                hid = sbp.tile([CLS_H, 128], b16, tag="hids")
                nc.scalar.activation(hid[:], hidp[:], Relu, bias=cb1[:, :1])
                logp = tpps.tile([N_CLASSES, 128], fp32, space="PSUM", tag="tp",
                                 name="logp")
                nc.tensor.matmul(out=logp[:], lhsT=cW2[:], rhs=hid[:],
                                 start=True, stop=True)
                nc.vector.tensor_tensor(
                    out=out_slab[:, bass.ts(gbv, 128)],
                    in0=cb2[:, 0:1].to_broadcast([N_CLASSES, 128]),
                    in1=logp[:], op=ADD)
            nc.sync.dma_start(logits_out[:], out_slab[:])

    nc.compile()
    return nc


def kernel(node_feats, src, dst, graph_ids,
           gW0, gb0, rW0, rb0, gW1, gb1, rW1, rb1, gW2, gb2, rW2, rb2,
           cW1, cb1, cW2, cb2):
    from concourse.bass_utils import run_bass_kernel_spmd

    node_feats = np.asarray(node_feats)
    cuts, gstart, per_core = _host_prep(node_feats, src, dst, graph_ids)

    if "nc" not in _cache:
        _cache["nc"] = _build_nc()
    nc = _cache["nc"]

    wvals = dict(
        gW0=gW0, rW0=rW0, gW1=gW1, rW1=rW1, gW2=gW2, rW2=rW2,
        cW1=cW1, cW2=cW2, gb0=gb0, rb0=rb0, gb1=gb1, rb1=rb1,
        gb2=gb2, rb2=rb2, cb1=cb1, cb2=cb2,
    )
    wblob = np.concatenate(
        [np.asarray(wvals[name], np.float32).reshape(-1) for name, _, _ in WSPECS])

    in_maps = [dict(h0=per_core[k]["h0"], meta=per_core[k]["meta"], wblob=wblob)
               for k in range(NCORES)]

    import time as _time
    _t0 = _time.perf_counter()
    res = run_bass_kernel_spmd(nc, in_maps, core_ids=list(range(NCORES)))
    _cache["last_run_wall_s"] = _time.perf_counter() - _t0

    out = np.zeros((N_GRAPHS, N_CLASSES), np.float32)
    for k in range(NCORES):
        ng = gstart[k + 1] - gstart[k]
        out[gstart[k]:gstart[k + 1]] = res.results[k]["logitsT"][:, :ng].T
    return out


# revision 15
# speedup vs baseline: 20.5916x; 1.0594x over previous
"""DGL-life GCN classifier on 8 Trainium2 NeuronCores (Bass/Tile).

Strategy: shard the batched graph by dst-node across 8 cores (cuts aligned to
graph boundaries).  The axon tunnel to the devices moves ~45 MB/s, so the
wire format is minimized: each core receives only its local fp8 feature
shard (the full table is built on-device with an AllGather), edge metadata is
packed one int32 word per edge slot (gather-row index | one-hot lane << 19),
and all weights ride in a single fp32 blob.  Local nodes are permuted
(serpentine deal by in-degree) so each 512-dst block needs exactly 64 gather
columns.  Per layer: per-edge rows are fetched with indirect DMA gathers,
segment-summed into 512-dst blocks via one-hot matmuls accumulated in PSUM,
then the dense GraphConv / residual transforms run in the transposed domain
where the per-output-feature bias rides the ACT relu for free.  Feature
shards are exchanged between layers with an AllGather.  SumPooling reuses
the same one-hot matmul machinery over graph ids, followed by the 2-layer
MLP classifier.
"""
import sys
sys.path.insert(0, "/opt/trn_rl_repo")

import numpy as np
import ml_dtypes

bf16 = ml_dtypes.bfloat16
fp8 = ml_dtypes.float8_e4m3

N_NODES = 500000
N_EDGES = 8000000
N_GRAPHS = 16384
IN_F = 74
HID = 64
CLS_H = 128
N_CLASSES = 2

NCORES = 8
NPAD = 62976          # padded nodes per shard (123 blocks of 512)
NB2 = 123             # 512-node dst blocks per shard (one For_i group each)
BW = 512              # dst-block width (one-hot lane count)
COLS2 = 64            # gather columns per block (8192 edge slots, balanced)
GRP = 4               # 128-row sub-tiles per block (residual transposes)
NTOT = NCORES * NPAD  # 503808 padded global rows
GPAD = 2176           # padded graphs per shard (17 blocks of 128)
NGB = GPAD // 128     # 17
PT = 33               # pooling node tiles per graph block
ETOT = NB2 * COLS2    # edge-meta columns (7872)
PTOT = NGB * PT       # pool-meta columns (561)
INVALID_E = 512 << 19  # lane 512 never matches iota 0..511
INVALID_P = 255 << 16  # lane 255 never matches iota 0..127

# weight blob layout: (name, rows, cols); m>1 entries become bf16 tiles
WSPECS = [
    ("gW0", IN_F, HID), ("rW0", IN_F, HID),
    ("gW1", HID, HID), ("rW1", HID, HID),
    ("gW2", HID, HID), ("rW2", HID, HID),
    ("cW1", HID, CLS_H), ("cW2", CLS_H, N_CLASSES),
    ("gb0", HID, 1), ("rb0", HID, 1), ("gb1", HID, 1), ("rb1", HID, 1),
    ("gb2", HID, 1), ("rb2", HID, 1), ("cb1", CLS_H, 1), ("cb2", N_CLASSES, 1),
]
WTOT = sum(k * m for _, k, m in WSPECS)

_cache = {}


def _host_prep(node_feats, src, dst, graph_ids):
    gid = np.asarray(graph_ids)
    node_of_graph_start = np.searchsorted(gid, np.arange(N_GRAPHS))
    cuts = [0]
    for k in range(1, NCORES):
        target = k * N_NODES // NCORES
        gi = np.searchsorted(node_of_graph_start, target)
        cand = [node_of_graph_start[min(gi, N_GRAPHS - 1)],
                node_of_graph_start[max(gi - 1, 0)]]
        cuts.append(int(min(cand, key=lambda x: abs(x - target))))
    cuts.append(N_NODES)
    cuts = np.asarray(cuts, np.int64)
    assert np.diff(cuts).max() <= NPAD

    gstart = [int(gid[c]) if c < N_NODES else N_GRAPHS for c in cuts[:-1]] + [N_GRAPHS]
    gstart = np.asarray(gstart, np.int64)
    assert np.diff(gstart).max() <= GPAD

    src = np.asarray(src).astype(np.int64)
    dst = np.asarray(dst).astype(np.int64)
    shard_of_dst = np.searchsorted(cuts, dst, side="right") - 1

    # pass 1: per-shard node permutation balancing edge load over 512-blocks
    # (serpentine deal of nodes sorted by in-degree)
    perms, masks = [], []
    perm_glob = np.empty(N_NODES, np.int64)
    for k in range(NCORES):
        base, n = cuts[k], cuts[k + 1] - cuts[k]
        m = shard_of_dst == k
        masks.append(m)
        indeg = np.bincount(dst[m] - base, minlength=n)
        order = np.argsort(-indeg, kind="stable")
        ids = np.arange(n)
        r, i = ids // NB2, ids % NB2
        bin_ = np.where(r % 2 == 0, i, NB2 - 1 - i)
        perm = np.empty(n, np.int64)
        perm[order] = bin_ * BW + r
        perms.append(perm)
        perm_glob[base:base + n] = k * NPAD + perm

    src_pad = perm_glob[src]

    per_core = []
    for k in range(NCORES):
        base, n = cuts[k], cuts[k + 1] - cuts[k]
        m, perm = masks[k], perms[k]
        e_src = src_pad[m]
        e_dst = perm[dst[m] - base]
        order = np.argsort(e_dst, kind="stable")
        e_src, e_dst = e_src[order], e_dst[order]
        blk = e_dst >> 9
        cnt = np.bincount(blk, minlength=NB2)
        assert cnt.max() <= COLS2 * 128, cnt.max()
        cum = np.concatenate([[0], np.cumsum(cnt)])
        slot = np.arange(len(e_dst)) - cum[blk]
        eword = np.full((NB2, COLS2 * 128), INVALID_E, np.int32)
        eword[blk, slot] = (e_src | ((e_dst & (BW - 1)) << 19)).astype(np.int32)
        eword = (eword.reshape(NB2, COLS2, 128)
                 .transpose(2, 0, 1).reshape(128, ETOT))

        # pooling: local nodes sorted by graph; graph-block-aligned slots
        gl = gid[base:base + n] - gstart[k]          # local graph id per node
        gb = gl >> 7
        pcnt = np.bincount(gb, minlength=NGB)
        assert pcnt.max() <= PT * 128
        pcum = np.concatenate([[0], np.cumsum(pcnt)])
        pslot = np.arange(n) - pcum[gb]
        pword = np.full((NGB, PT * 128), INVALID_P, np.int32)
        pword[gb, pslot] = (perm | ((gl & 127) << 16)).astype(np.int32)
        pword = (pword.reshape(NGB, PT, 128)
                 .transpose(2, 0, 1).reshape(128, PTOT))

        meta = np.concatenate([eword, pword], axis=1)

        h0 = np.zeros((NPAD, IN_F), fp8)
        h0[perm] = node_feats[base:base + n].astype(fp8)

        per_core.append(dict(meta=np.ascontiguousarray(meta), h0=h0))
    return cuts, gstart, per_core


def _build_nc():
    import concourse.bass as bass
    from concourse import bacc
    import concourse.mybir as mybir
    import concourse.tile as tile

    fp32 = mybir.dt.float32
    b16 = mybir.dt.bfloat16
    f8 = mybir.dt.float8e4
    i32 = mybir.dt.int32

    nc = bacc.Bacc("TRN2", target_bir_lowering=False, debug=False,
                   num_devices=NCORES)

    h0_in = nc.dram_tensor("h0", [NPAD, IN_F], f8, kind="ExternalInput")
    meta_in = nc.dram_tensor("meta", [128, ETOT + PTOT], i32, kind="ExternalInput")
    wblob_in = nc.dram_tensor("wblob", [WTOT], fp32, kind="ExternalInput")
    logits_out = nc.dram_tensor("logitsT", [N_CLASSES, GPAD], fp32, kind="ExternalOutput")

    emeta_ap = meta_in[:, 0:ETOT].rearrange("p (g c) -> p g c", c=COLS2)
    pmeta_ap = meta_in[:, ETOT:ETOT + PTOT].rearrange("p (g c) -> p g c", c=PT)
    # L0 residual rhs source: [g, b, 128, f] view of the local shard
    h0r_ap = h0_in[:].rearrange("(g b p) f -> g b p f", b=GRP, p=128)

    Relu = mybir.ActivationFunctionType.Relu
    EQ = mybir.AluOpType.is_equal
    AND = mybir.AluOpType.bitwise_and
    SHR = mybir.AluOpType.logical_shift_right
    ADD = mybir.AluOpType.add

    with tile.TileContext(nc) as tc:
        with (
            tc.tile_pool(name="const", bufs=1) as constp,
            tc.tile_pool(name="persist", bufs=1) as persistp,
            tc.tile_pool(name="meta", bufs=2) as metap,
            tc.tile_pool(name="slab", bufs=2) as slabp,
            tc.tile_pool(name="p1", bufs=4) as pp,
            tc.tile_pool(name="sb", bufs=4) as sbp,
            tc.tile_pool(name="agg_ps", bufs=2, space="PSUM") as aggps,
            tc.tile_pool(name="mm_ps", bufs=2, space="PSUM") as mmps,
            tc.tile_pool(name="tp_ps", bufs=2, space="PSUM") as tpps,
            tc.tile_pool(name="dram", bufs=1, space="DRAM") as dramp,
        ):
            # on-device constants: iota row (512-wide) + bf16 identity
            iota_i = constp.tile([128, BW], i32)
            nc.gpsimd.iota(iota_i[:], pattern=[[1, BW]], base=0,
                           channel_multiplier=0)
            ones = constp.tile([128, 128], b16)
            nc.vector.memset(ones[:], 1.0)
            ident = constp.tile([128, 128], b16)
            nc.gpsimd.affine_select(out=ident[:], in_=ones[:],
                                    pattern=[[-1, 128]], compare_op=EQ,
                                    fill=0.0, base=0, channel_multiplier=1)

            # weights from the blob: matmul weights cast to bf16 via SWDGE,
            # biases stay fp32
            wt = {}
            off = 0
            for name, k, m in WSPECS:
                ap = wblob_in[off:off + k * m].rearrange("(k m) -> k m", m=m)
                if m > 1:
                    t = constp.tile([k, m], b16, tag=f"w16_{name}",
                                    name=f"w16_{name}")
                    nc.gpsimd.dma_start(t[:], ap)
                else:
                    t = constp.tile([k, 1], fp32, tag=f"b_{name}",
                                    name=f"b_{name}")
                    nc.sync.dma_start(t[:], ap)
                wt[name] = t
                off += k * m
            gW = [wt["gW0"], wt["gW1"], wt["gW2"]]
            rW = [wt["rW0"], wt["rW1"], wt["rW2"]]
            gb = [wt["gb0"], wt["gb1"], wt["gb2"]]
            rb = [wt["rb0"], wt["rb1"], wt["rb2"]]
            cW1, cW2, cb1, cb2 = wt["cW1"], wt["cW2"], wt["cb1"], wt["cb2"]

            # DRAM scratch: full feature tables (AllGathered), local
            # transposed features, final local features
            table0 = dramp.tile([NTOT, IN_F], f8, addr_space="Shared",
                                name="table0")
            cc_in = [dramp.tile([NPAD, HID], b16, name=f"cc_in{i}") for i in range(2)]
            cc_out = [dramp.tile([NTOT, HID], b16, addr_space="Shared",
                                 name=f"cc_out{i}") for i in range(2)]
            hT_dram = dramp.tile([HID, NPAD], b16, name="hT_dram")
            h3_local = dramp.tile([NPAD, HID], b16, name="h3_local")

            h0_stage = dramp.tile([NPAD, IN_F], f8, name="h0_stage")
            nc.sync.dma_start(h0_stage[:], h0_in[:])
            nc.gpsimd.collective_compute(
                "AllGather", mybir.AluOpType.bypass,
                replica_groups=[list(range(NCORES))],
                ins=[h0_stage[:].opt()], outs=[table0[:].opt()])

            for L in range(3):
                DIN = IN_F if L == 0 else HID
                table_ap = table0[:] if L == 0 else cc_out[L - 1][:]
                with tc.For_i(0, NB2, 1) as g:
                    w_t = metap.tile([128, COLS2], i32, tag="w")
                    nc.sync.dma_start(w_t[:], emeta_ap[:, bass.ds(g, 1), :])
                    idx_t = metap.tile([128, COLS2], i32, tag="idx")
                    nc.vector.tensor_scalar(out=idx_t[:], in0=w_t[:],
                                            scalar1=0x7FFFF, scalar2=None,
                                            op0=AND)
                    lane_t = metap.tile([128, COLS2], i32, tag="lane")
                    nc.vector.tensor_scalar(out=lane_t[:], in0=w_t[:],
                                            scalar1=19, scalar2=None, op0=SHR)

                    if L == 0:
                        slab = slabp.tile([128, COLS2, IN_F], f8, tag="slab8",
                                          name="slab8")[:, :, :DIN]
                    else:
                        slab = slabp.tile([128, COLS2, IN_F], b16, tag="slab",
                                          name="slab")[:, :, :DIN]
                    for j in range(COLS2):
                        nc.gpsimd.indirect_dma_start(
                            out=slab[:, j, :], out_offset=None, in_=table_ap,
                            in_offset=bass.IndirectOffsetOnAxis(
                                ap=idx_t[:, j:j + 1], axis=0))

                    # residual rhs: transposed local features for this block
                    hTg = slabp.tile([IN_F, BW], b16, tag="hTg",
                                     name="hTg")[:DIN, :]
                    if L == 0:
                        h0blk8 = slabp.tile([128, GRP * IN_F], f8, tag="h0blk8")
                        for b6 in range(GRP):
                            nc.sync.dma_start(
                                h0blk8[:, b6 * IN_F:(b6 + 1) * IN_F],
                                h0r_ap[bass.ds(g, 1), b6:b6 + 1, :, :])
                        h0blk = slabp.tile([128, GRP * IN_F], b16, tag="h0blk")
                        nc.vector.tensor_copy(h0blk[:], h0blk8[:])
                        for b6 in range(GRP):
                            tp0 = tpps.tile([IN_F, 128], b16, space="PSUM",
                                            tag="tp")
                            nc.tensor.transpose(
                                out=tp0[:], in_=h0blk[:, b6 * IN_F:(b6 + 1) * IN_F],
                                identity=ident[:])
                            nc.vector.tensor_copy(
                                hTg[:, b6 * 128:(b6 + 1) * 128], tp0[:])
                    else:
                        nc.sync.dma_start(hTg[:], hT_dram[:, bass.ts(g, BW)])

                    # segment-sum via one-hot matmuls into one 512-wide psum
                    psum = aggps.tile([DIN, BW], fp32, space="PSUM", tag="agg")
                    for j in range(COLS2):
                        if L == 0:
                            p_t = pp.tile([128, BW], f8, tag="p8", name="p8")
                        else:
                            p_t = pp.tile([128, BW], b16, tag="p16", name="p16")
                        nc.vector.tensor_tensor(
                            out=p_t[:],
                            in0=lane_t[:, j:j + 1].to_broadcast([128, BW]),
                            in1=iota_i[:], op=EQ)
                        nc.tensor.matmul(out=psum[:], lhsT=slab[:, j, :],
                                         rhs=p_t[:], start=(j == 0),
                                         stop=(j == COLS2 - 1))
                    aggT = sbp.tile([DIN, BW], b16, tag="aggT")
                    nc.vector.tensor_copy(aggT[:], psum[:])
                    convp = mmps.tile([HID, BW], fp32, space="PSUM", tag="conv")
                    nc.tensor.matmul(out=convp[:], lhsT=gW[L][:], rhs=aggT[:],
                                     start=True, stop=True)
                    resp = mmps.tile([HID, BW], fp32, space="PSUM", tag="res")
                    nc.tensor.matmul(out=resp[:], lhsT=rW[L][:], rhs=hTg[:],
                                     start=True, stop=True)
                    convs = sbp.tile([HID, BW], b16, tag="convs")
                    nc.scalar.activation(convs[:], convp[:], Relu, bias=gb[L][:, :1])
                    ress = sbp.tile([HID, BW], b16, tag="ress")
                    nc.scalar.activation(ress[:], resp[:], Relu, bias=rb[L][:, :1])
                    hnewT = sbp.tile([HID, BW], b16, tag="hnewT")
                    nc.vector.tensor_add(hnewT[:], convs[:], ress[:])
                    if L < 2:
                        nc.sync.dma_start(hT_dram[:, bass.ts(g, BW)], hnewT[:])
                    dst_dram = cc_in[L] if L < 2 else h3_local
                    dd = dst_dram[:].rearrange("(g x) d -> g x d", x=BW)
                    for b6 in range(GRP):
                        tp = tpps.tile([128, HID], b16, space="PSUM", tag="tp")
                        nc.tensor.transpose(
                            out=tp[:], in_=hnewT[:, b6 * 128:(b6 + 1) * 128],
                            identity=ident[:HID, :HID])
                        hnew = sbp.tile([128, HID], b16, tag="hnew")
                        nc.vector.tensor_copy(hnew[:], tp[:])
                        nc.sync.dma_start(
                            dd[bass.ds(g, 1), b6 * 128:(b6 + 1) * 128, :], hnew[:])
                if L < 2:
                    nc.gpsimd.collective_compute(
                        "AllGather", mybir.AluOpType.bypass,
                        replica_groups=[list(range(NCORES))],
                        ins=[cc_in[L][:].opt()], outs=[cc_out[L][:].opt()])

            # -------- pooling + classifier --------
            out_slab = persistp.tile([N_CLASSES, GPAD], fp32)
            with tc.For_i(0, NGB, 1) as gbv:
                wp_t = metap.tile([128, PT], i32, tag="wp")
                nc.sync.dma_start(wp_t[:], pmeta_ap[:, bass.ds(gbv, 1), :])
                pidx_t = metap.tile([128, PT], i32, tag="pidx")
                nc.vector.tensor_scalar(out=pidx_t[:], in0=wp_t[:],
                                        scalar1=0xFFFF, scalar2=None, op0=AND)
                plane_t = metap.tile([128, PT], i32, tag="plane")
                nc.vector.tensor_scalar(out=plane_t[:], in0=wp_t[:],
                                        scalar1=16, scalar2=None, op0=SHR)
                pslab = slabp.tile([128, PT, HID], b16, tag="pslab")
                for t in range(PT):
                    nc.gpsimd.indirect_dma_start(
                        out=pslab[:, t, :], out_offset=None, in_=h3_local[:],
                        in_offset=bass.IndirectOffsetOnAxis(
                            ap=pidx_t[:, t:t + 1], axis=0))
                gpsum = aggps.tile([HID, 128], fp32, space="PSUM", tag="agg",
                                   name="gpsum")
                for t in range(PT):
                    p_t = pp.tile([128, 128], b16, tag="pp")
                    nc.vector.tensor_tensor(
                        out=p_t[:], in0=plane_t[:, t:t + 1].to_broadcast([128, 128]),
                        in1=iota_i[:, 0:128], op=EQ)
                    nc.tensor.matmul(out=gpsum[:], lhsT=pslab[:, t, :], rhs=p_t[:],
                                     start=(t == 0), stop=(t == PT - 1))
                graphT = sbp.tile([HID, 128], b16, tag="graphT")
                nc.vector.tensor_copy(graphT[:], gpsum[:])
                hidp = mmps.tile([CLS_H, 128], fp32, space="PSUM", tag="conv",
                                 name="hidp")
                nc.tensor.matmul(out=hidp[:], lhsT=cW1[:], rhs=graphT[:],
                                 start=True, stop=True)
                hid = sbp.tile([CLS_H, 128], b16, tag="hids")
                nc.scalar.activation(hid[:], hidp[:], Relu, bias=cb1[:, :1])
                logp = tpps.tile([N_CLASSES, 128], fp32, space="PSUM", tag="tp",
                                 name="logp")
                nc.tensor.matmul(out=logp[:], lhsT=cW2[:], rhs=hid[:],
                                 start=True, stop=True)
                nc.vector.tensor_tensor(
                    out=out_slab[:, bass.ts(gbv, 128)],
                    in0=cb2[:, 0:1].to_broadcast([N_CLASSES, 128]),
                    in1=logp[:], op=ADD)
            nc.sync.dma_start(logits_out[:], out_slab[:])

    nc.compile()
    return nc


def kernel(node_feats, src, dst, graph_ids,
           gW0, gb0, rW0, rb0, gW1, gb1, rW1, rb1, gW2, gb2, rW2, rb2,
           cW1, cb1, cW2, cb2):
    from concourse.bass_utils import run_bass_kernel_spmd

    node_feats = np.asarray(node_feats)
    cuts, gstart, per_core = _host_prep(node_feats, src, dst, graph_ids)

    if "nc" not in _cache:
        _cache["nc"] = _build_nc()
    nc = _cache["nc"]

    wvals = dict(
        gW0=gW0, rW0=rW0, gW1=gW1, rW1=rW1, gW2=gW2, rW2=rW2,
        cW1=cW1, cW2=cW2, gb0=gb0, rb0=rb0, gb1=gb1, rb1=rb1,
        gb2=gb2, rb2=rb2, cb1=cb1, cb2=cb2,
    )
    wblob = np.concatenate(
        [np.asarray(wvals[name], np.float32).reshape(-1) for name, _, _ in WSPECS])

    in_maps = [dict(h0=per_core[k]["h0"], meta=per_core[k]["meta"], wblob=wblob)
               for k in range(NCORES)]

    import time as _time
    _t0 = _time.perf_counter()
    res = run_bass_kernel_spmd(nc, in_maps, core_ids=list(range(NCORES)))
    _cache["last_run_wall_s"] = _time.perf_counter() - _t0

    out = np.zeros((N_GRAPHS, N_CLASSES), np.float32)
    for k in range(NCORES):
        ng = gstart[k + 1] - gstart[k]
        out[gstart[k]:gstart[k + 1]] = res.results[k]["logitsT"][:, :ng].T
    return out
